# revision 1
# baseline (speedup 1.0000x reference)
"""Multi-head attention (B=2, S=4096, D=512, H=8) on 8 TRN2 NeuronCores.

Sharding: (batch, head-pair) tensor parallel. Core i handles batch i//4
and heads 2*(i%4), 2*(i%4)+1. Each core computes Q/K/V projections only
for its two heads (1/4 of the projection work, no redundancy), full
S x S attention for those heads over all 4096 queries, and a PARTIAL
output projection out_partial = aot_pair^T @ Wo_pair + bo/4. The host
sums the 4 partials per batch (f32) -- no device collectives.

Per-core device pipeline (d-major transposed layout, bf16 matmuls):
  1. Transposing DMAs load x^T [d, t] in 4 segments of 1024 t.
  2. Q^T/K^T = Wpair x^T (f=512 matmuls, 128-row output = both heads);
     V natural [t, pair-dv] with a ones-column per head (V_aug).
  3. Per (q-tile 512, k-chunk 128): 2 row-packed score matmuls
     (c=64, heads at PE rows 0-63/64-127 run concurrently), one ACT
     exp [128,1024] psum->sbuf (scale=1/8), 2 attn@V matmuls
     lhsT=[V_h|1] [128,65] -> po [65,512]; row 64 accumulates the
     softmax denominator. scores/exp for k+1 are emitted before attn@V
     of k (software pipeline) so ACT -- the bottleneck engine -- is
     never starved.
  4. Normalize per q-tile: copy numerators into one partition-aligned
     [128, 512] sbuf tile and denominators to partition-0 rows (custom
     DVE ops require base partition 0), fast-approx reciprocal, two
     fp16 rank-1 broadcast matmuls into one psum bank, one aligned
     scalar_tensor_tensor multiply -> aot [128(2 heads' d), 512].
  5. Partial output projection: one c=128 matmul per 128-row t-chunk
     (both heads contracted at once), + bo/4, DMA out f32.

Steady state is ACT(exp)-bound: 256 instrs x ~1.0us ~= 260us; PE has
~80us of slack which absorbs projections and normalization.
"""

import numpy as np
import ml_dtypes

import concourse.bass as bass
import concourse.tile as tile
from concourse import bacc, mybir
from concourse.bass_utils import run_bass_kernel_spmd

F32 = mybir.dt.float32
F32R = mybir.dt.float32r
FP16 = mybir.dt.float16
BF16 = mybir.dt.bfloat16
MUL = mybir.AluOpType.mult

B, S, D, H = 2, 4096, 512, 8
HD = D // H  # 64
NCORES = 8
PAIRS = 4  # head-pairs; one per core (per batch)
IC = D // 128  # 4 contraction chunks over d_model
QT = 512  # q tile
NQT = S // QT  # 8
KCH = S // 128  # 32 k chunks
SEG = 1024  # t-columns per transposed DMA segment
NSEG = S // SEG  # 4


def _build_program():
    nc = bacc.Bacc(
        "TRN2",
        target_bir_lowering=False,
        debug=False,
        enable_asserts=False,
        num_devices=NCORES,
    )
    xt = nc.dram_tensor("xt", [D, S], BF16, kind="ExternalInput").ap()
    wqt = nc.dram_tensor("wqt", [D, 128], BF16, kind="ExternalInput").ap()
    wkt = nc.dram_tensor("wkt", [D, 128], BF16, kind="ExternalInput").ap()
    wvt = nc.dram_tensor("wvt", [D, 128], BF16, kind="ExternalInput").ap()
    wos = nc.dram_tensor("wos", [128, D], BF16, kind="ExternalInput").ap()
    bqs = nc.dram_tensor("bqs", [128, 1], F32, kind="ExternalInput").ap()
    bks = nc.dram_tensor("bks", [128, 1], F32, kind="ExternalInput").ap()
    bvb = nc.dram_tensor("bvb", [128, 128], F32, kind="ExternalInput").ap()
    bob = nc.dram_tensor("bob", [128, D], F32, kind="ExternalInput").ap()
    out = nc.dram_tensor("out", [S, D], F32, kind="ExternalOutput").ap()

    with tile.TileContext(nc) as tc:
        with (
            tc.tile_pool(name="consts", bufs=1) as consts,
            tc.tile_pool(name="persist", bufs=1) as persist,
            tc.tile_pool(name="pt", bufs=6) as pt_pool,
            tc.tile_pool(name="aot", bufs=2) as aot_pool,
            tc.tile_pool(name="osb", bufs=4) as osb_pool,
            tc.tile_pool(name="posb", bufs=4) as posb_pool,
            tc.tile_pool(name="small", bufs=4) as small_pool,
            # PSUM (8 banks): sc 2x2, po 2x1, acc 2x1
            tc.tile_pool(name="ps_sc", bufs=2, space="PSUM") as sc_pool,
            tc.tile_pool(name="ps_po", bufs=2, space="PSUM") as po_pool,
            tc.tile_pool(name="ps_acc", bufs=2, space="PSUM") as acc_pool,
        ):
            # ---- constants ----
            ones64f = consts.tile([1, HD], F32)
            nc.vector.memset(ones64f, 1.0)
            ones64 = consts.tile([1, HD], FP16)
            nc.vector.tensor_copy(ones64, ones64f)

            # ---- persistent activations ----
            xtks = [
                persist.tile([128, IC, SEG], BF16, name=f"xtk{s}")
                for s in range(NSEG)
            ]
            kt = persist.tile([128, S], BF16)  # K^T pair [dv, t]
            qt = persist.tile([128, S], BF16)  # Q^T pair
            # V_aug: [t-in-chunk, t-chunk, head-in-pair, 64 V cols + ones]
            v_sb = persist.tile([128, KCH, 2, HD + 1], BF16)
            nc.vector.memset(v_sb[:, :, :, HD : HD + 1], 1.0)

            # ---- DMAs: x^T is pre-transposed on the host, so these are
            # plain (non-XBAR) loads: seg 0, then weights, then segs 1-3.
            xtd = xt.rearrange("(c p) t -> p c t", p=128)
            nc.sync.dma_start(xtks[0][:, :, 0:512], xtd[:, :, 0:512])
            # everything the upfront projection units need (they only read
            # t 0..511) goes right after the first half-segment; the second
            # half and later segments follow
            wq_sb = consts.tile([128, IC, 128], BF16)
            nc.sync.dma_start(wq_sb, wqt.rearrange("(c p) o -> p c o", p=128))
            wk_sb = consts.tile([128, IC, 128], BF16)
            nc.sync.dma_start(wk_sb, wkt.rearrange("(c p) o -> p c o", p=128))
            bq_sb = consts.tile([128, 1], F32)
            nc.sync.dma_start(bq_sb, bqs)
            bk_sb = consts.tile([128, 1], F32)
            nc.sync.dma_start(bk_sb, bks)
            wv_sb = consts.tile([128, IC, 128], BF16)
            nc.sync.dma_start(wv_sb, wvt.rearrange("(c p) o -> p c o", p=128))
            bvb_sb = consts.tile([128, 128], F32)
            nc.sync.dma_start(bvb_sb, bvb)
            nc.sync.dma_start(xtks[0][:, :, 512:SEG], xtd[:, :, 512:SEG])
            bob_sb = consts.tile([128, D], F32)
            nc.sync.dma_start(bob_sb, bob)
            wo_sb = consts.tile([128, D], BF16)
            nc.sync.dma_start(wo_sb, wos)
            for s in range(1, NSEG):
                nc.sync.dma_start(xtks[s], xtd[:, :, s * SEG : (s + 1) * SEG])

            # ---- projection units ----
            def q_unit(tt):
                ps = acc_pool.tile([128, QT], F32, tag="acc", name=f"q{tt}")
                s, ss = divmod(tt, 2)
                for i in range(IC):
                    nc.tensor.matmul(
                        ps,
                        wq_sb[:, i, :],
                        xtks[s][:, i, ss * QT : (ss + 1) * QT],
                        start=(i == 0),
                        stop=(i == IC - 1),
                    )
                nc.vector.tensor_scalar_add(
                    qt[:, tt * QT : (tt + 1) * QT], ps, bq_sb[:, 0:1]
                )

            def k_unit(tt, lo=0, hi=QT):
                ps = acc_pool.tile(
                    [128, hi - lo], F32, tag="acc", name=f"k{tt}_{lo}"
                )
                s, ss = divmod(tt, 2)
                for i in range(IC):
                    nc.tensor.matmul(
                        ps,
                        wk_sb[:, i, :],
                        xtks[s][:, i, ss * QT + lo : ss * QT + hi],
                        start=(i == 0),
                        stop=(i == IC - 1),
                    )
                nc.vector.tensor_scalar_add(
                    kt[:, tt * QT + lo : tt * QT + hi], ps, bk_sb[:, 0:1]
                )

            def v_unit(j):
                # V rows for t-chunk j, both heads: [128 t, 128 dv] + bias
                ps = acc_pool.tile([128, 128], F32, tag="acc", name=f"v{j}")
                s, jj = divmod(j, 8)
                for i in range(IC):
                    nc.tensor.matmul(
                        ps,
                        xtks[s][:, i, jj * 128 : (jj + 1) * 128],
                        wv_sb[:, i, :],
                        start=(i == 0),
                        stop=(i == IC - 1),
                    )
                nc.vector.tensor_add(
                    v_sb[:, j, :, 0:HD],
                    ps.rearrange("p (h d) -> p h d", h=2),
                    bvb_sb.rearrange("p (h d) -> p h d", h=2),
                )

            # upfront: just enough for attention (qi=0) to start; K tile 0
            # is split so chunk 0's scores wait only on a 128-col mini-unit
            q_unit(0)
            k_unit(0, 0, 128)
            v_unit(0)
            k_unit(0, 128, QT)
            v_unit(1)
            k_unit(1)
            v_unit(2)
            v_unit(3)
            pending = []
            for u in (4, 5, 6, 7):
                pending.append(lambda j=u: v_unit(j))
            pending.append(lambda: q_unit(1))
            for tt in range(2, 8):  # k segs with their v chunks
                pending.append(lambda tt=tt: k_unit(tt))
                for j in range(4 * tt, 4 * tt + 4):
                    pending.append(lambda j=j: v_unit(j))
            for tt in range(2, 8):
                pending.append(lambda tt=tt: q_unit(tt))

            pending_slow = []

            aots = {}

            def norm_unit(qi, posbN, recs):
                pb2 = acc_pool.tile([128, QT], F32, tag="acc", name=f"pb{qi}")
                nc.tensor.matmul(
                    pb2[0:HD, :], ones64, recs[0], start=True, stop=True
                )
                nc.tensor.matmul(
                    pb2[HD:128, :], ones64, recs[1], start=True, stop=True
                )
                nc.vector.scalar_tensor_tensor(
                    aots[qi], pb2, 1.0, posbN, op0=MUL, op1=MUL
                )

            def fin_unit(qi, t4):
                ps = acc_pool.tile([128, D], F32, tag="acc", name=f"f{qi}_{t4}")
                nc.tensor.matmul(
                    ps,
                    aots[qi][:, t4 * 128 : (t4 + 1) * 128],
                    wo_sb,
                    start=True,
                    stop=True,
                )
                osb = osb_pool.tile([128, D], F32, tag="osb")
                nc.vector.tensor_add(osb, ps, bob_sb)
                t0 = qi * QT + t4 * 128
                nc.sync.dma_start(out[t0 : t0 + 128, :], osb)

            # ---- attention ----
            for qi in range(NQT):
                qs = qi * QT
                aots[qi] = aot_pool.tile(
                    [128, QT], BF16, tag="aot", name=f"aot{qi}"
                )
                po = [
                    po_pool.tile([HD + 1, QT], F32, tag="po", name=f"po{qi}_{hh}")
                    for hh in range(2)
                ]

                def scores_exp(k, qs=qs):
                    pss = sc_pool.tile([128, 2, QT], F32, tag="sc")
                    for hh in range(2):
                        off = hh * HD
                        nc.tensor.matmul(
                            pss[:, hh, :],
                            kt[off : off + HD, k * 128 : (k + 1) * 128],
                            qt[off : off + HD, qs : qs + QT],
                            start=True,
                            stop=True,
                        )
                    ptile = pt_pool.tile([128, 2, QT], BF16, tag="pt")
                    nc.scalar.activation(
                        ptile, pss, mybir.ActivationFunctionType.Exp,
                        scale=1.0 / np.sqrt(HD),
                    )
                    return ptile

                # software pipeline: scores/exp for k+1 before attn@V of k
                ptile = scores_exp(0)
                for k in range(KCH):
                    it = qi * KCH + k
                    nxt = scores_exp(k + 1) if k + 1 < KCH else None
                    for hh in range(2):
                        nc.tensor.matmul(
                            po[hh],
                            v_sb[:, k, hh, :],
                            ptile[:, hh, :],
                            start=(k == 0),
                            stop=(k == KCH - 1),
                        )
                    ptile = nxt
                    # drain deferred work into PE's slack (~1.5 units/iter
                    # keeps proj ahead of its deadlines without bursts that
                    # starve ACT)
                    if it >= 1 and pending and (
                        len(pending) > 6 or it % 4 == 2
                    ):
                        pending.pop(0)()
                        if it % 2 == 0 and pending and len(pending) > 6:
                            pending.pop(0)()
                    elif it % 5 == 0 and pending_slow:
                        pending_slow.pop(0)()

                # free po banks fast; defer the slow normalize + fin chain.
                # For the last q-tile, the numerator copies run on ACT (idle
                # after the final exp) in parallel with the DVE reciprocal
                # chain to shorten the tail.
                posbN = posb_pool.tile([128, QT], F32, tag="posb", name=f"posb{qi}")
                last = qi == NQT - 1
                if not last:
                    nc.vector.tensor_copy(posbN[0:HD, :], po[0][0:HD, :])
                    nc.vector.tensor_copy(posbN[HD : 2 * HD, :], po[1][0:HD, :])
                recs = []
                for hh in range(2):
                    db = small_pool.tile([1, QT], F32, tag="db")
                    if last and hh == 1:
                        # tail: ACT takes one denominator copy so the two
                        # reciprocal chains overlap across engines
                        nc.scalar.copy(db, po[hh][HD : HD + 1, :])
                    else:
                        nc.vector.tensor_copy(db, po[hh][HD : HD + 1, :])
                    recf = small_pool.tile([1, QT], F32, tag="recf")
                    nc.vector.reciprocal_approx_fast(recf, db)
                    rec = small_pool.tile([1, QT], FP16, tag="rec")
                    if last:
                        nc.scalar.copy(rec, recf)
                    else:
                        nc.vector.tensor_copy(rec, recf)
                    recs.append(rec)
                if last:
                    nc.scalar.copy(posbN[0:HD, :], po[0][0:HD, :])
                    nc.scalar.copy(posbN[HD : 2 * HD, :], po[1][0:HD, :])
                pending_slow.append(
                    lambda qi=qi, posbN=posbN, recs=recs: norm_unit(qi, posbN, recs)
                )
                pending_slow.extend(
                    lambda qi=qi, t4=t4: fin_unit(qi, t4) for t4 in range(4)
                )

            for u in pending + pending_slow:
                u()

    nc.compile()
    return nc


_NC_CACHE = None


def _get_program():
    global _NC_CACHE
    if _NC_CACHE is None:
        _NC_CACHE = _build_program()
    return _NC_CACHE


def prepare_in_maps(x, Wq, bq, Wk, bk, Wv, bv, Wo, bo):
    bf = ml_dtypes.bfloat16
    x = np.ascontiguousarray(np.asarray(x, dtype=np.float32)).astype(bf)
    wqT = np.asarray(Wq, np.float32).T  # [D in, D out-rows]
    wkT = np.asarray(Wk, np.float32).T
    wvT = np.asarray(Wv, np.float32).T
    woT = np.asarray(Wo, np.float32).T  # [D dv, D out]
    bq = np.asarray(bq, np.float32)
    bk = np.asarray(bk, np.float32)
    bv = np.asarray(bv, np.float32)
    bo = np.asarray(bo, np.float32)
    in_maps = []
    for core in range(NCORES):
        b = core // PAIRS
        hp = core % PAIRS
        pr = slice(hp * 128, (hp + 1) * 128)
        m = {
            "xt": np.ascontiguousarray(x[b].T),
            "wqt": np.ascontiguousarray(wqT[:, pr]).astype(bf),
            "wkt": np.ascontiguousarray(wkT[:, pr]).astype(bf),
            "wvt": np.ascontiguousarray(wvT[:, pr]).astype(bf),
            "wos": np.ascontiguousarray(woT[pr, :]).astype(bf),
            "bqs": np.ascontiguousarray(bq[pr].reshape(128, 1)),
            "bks": np.ascontiguousarray(bk[pr].reshape(128, 1)),
            "bvb": np.ascontiguousarray(
                np.broadcast_to(bv[pr][None, :], (128, 128))
            ),
            "bob": np.ascontiguousarray(
                np.broadcast_to(bo[None, :] * 0.25, (128, D))
            ),
        }
        in_maps.append(m)
    return in_maps


def assemble(results):
    out = np.empty((B, S, D), dtype=np.float32)
    for b in range(B):
        acc = results[b * PAIRS]["out"].astype(np.float32, copy=True)
        for hp in range(1, PAIRS):
            acc += results[b * PAIRS + hp]["out"]
        out[b] = acc
    return out


def kernel(x, Wq, bq, Wk, bk, Wv, bv, Wo, bo):
    in_maps = prepare_in_maps(x, Wq, bq, Wk, bk, Wv, bv, Wo, bo)
    nc = _get_program()
    res = run_bass_kernel_spmd(nc, in_maps, core_ids=list(range(NCORES)))
    return assemble(res.results)



# revision 4
# speedup vs baseline: 1.0346x; 1.0346x over previous
"""Multi-head attention (B=2, S=4096, D=512, H=8) on 8 TRN2 NeuronCores.

Sharding: (batch, head-pair) tensor parallel. Core i handles batch i//4
and heads 2*(i%4), 2*(i%4)+1. Each core computes Q/K/V projections only
for its two heads, full S x S attention for those heads, and a PARTIAL
output projection. The host sums the 4 partials per batch (f32) and adds
bo once -- no device collectives.

Key structure (v2): the softmax exp -- the baseline bottleneck at
256 x 1.09us on ACT -- is split across TWO engines, and attn@V runs in
fp8 DoubleRow (contraction 256 = two 128-k-chunks per matmul):

  1. Transposing host prep: x^T loaded [d, t] in 4 segments.
  2. Q^T/K^T = W x^T bf16 (both heads row-packed); V -> fp8e4 V_aug
     [t, 2-chunk-pair, 65] with a ones column (softmax denominator).
  3. Per (q-tile 512, k-chunk 128): 2 row-tiled score matmuls (c=64,
     heads at PE rows 0-63/64-127 run concurrently) -> psum [128,2,512].
     exp: EVEN chunks on ACT (exact exp -> fp8e4 convert); ODD chunks on
     DVE as a single tensor_scalar (x*log2e + 56.05 -> uint8) that
     constructs the e4m3 BIT PATTERN directly (Schraudolph in fp8
     space; constant tuned so the path is bias-free vs exact exp).
  4. attn@V: per (chunk-PAIR, head) one DoubleRow fp8 matmul
     lhsT=[128,2,65] V_aug-pair, rhs=[128,2,512] p-pair, accumulating
     po [65, 1024] (both heads in one 2-bank psum tile; row 64 = the
     softmax denominators).
  5. Normalize per q-tile: denominators [1,1024] -> fast reciprocal ->
     fp16 -> two rank-1 broadcast matmuls -> one scalar_tensor_tensor
     multiply -> aot bf16. Output projection per 128-t chunk, f32 out.
     bo is added on the host during the gather.

Steady state: PE ~scores 512cyc + attnV-DR ~640cyc per chunk-pair;
ACT ~1.09us/even-chunk exp; DVE ~1.19us/odd-chunk exp; aux ops spread
across ACT/DVE for balance.
"""

import numpy as np
import ml_dtypes

import concourse.bass as bass
import concourse.tile as tile
from concourse import bacc, mybir
from concourse.bass_utils import run_bass_kernel_spmd

F32 = mybir.dt.float32
FP16 = mybir.dt.float16
BF16 = mybir.dt.bfloat16
FP8 = mybir.dt.float8e4
U8 = mybir.dt.uint8
MUL = mybir.AluOpType.mult
ADD = mybir.AluOpType.add
DR = mybir.MatmulPerfMode.DoubleRow

B, S, D, H = 2, 4096, 512, 8
HD = D // H  # 64
NCORES = 8
PAIRS = 4  # head-pairs; one per core (per batch)
IC = D // 128  # 4 contraction chunks over d_model
QT = 512  # q tile
NQT = S // QT  # 8
KCH = S // 128  # 32 k chunks
NCP = KCH // 2  # 16 chunk pairs (DoubleRow contracts 2 chunks)
SEG = 1024  # t-columns per transposed DMA segment
NSEG = S // SEG  # 4

LOG2E = 1.4426950408889634
# e4m3 Schraudolph bias: 56 (exponent bias*8) + sawtooth centering +0.5
# for the truncating f32->uint8 convert. Tuned numerically for zero
# multiplicative bias vs the exact-exp path (see session notes).
SCHRAU_C = 56.05


def _build_program():
    nc = bacc.Bacc(
        "TRN2",
        target_bir_lowering=False,
        debug=False,
        enable_asserts=False,
        num_devices=NCORES,
    )
    xt = nc.dram_tensor("xt", [D, S], BF16, kind="ExternalInput").ap()
    wqt = nc.dram_tensor("wqt", [D, 128], BF16, kind="ExternalInput").ap()
    wkt = nc.dram_tensor("wkt", [D, 128], BF16, kind="ExternalInput").ap()
    wvt = nc.dram_tensor("wvt", [D, 128], BF16, kind="ExternalInput").ap()
    wos = nc.dram_tensor("wos", [128, D], BF16, kind="ExternalInput").ap()
    bqs = nc.dram_tensor("bqs", [128, 1], F32, kind="ExternalInput").ap()
    bks = nc.dram_tensor("bks", [128, 1], F32, kind="ExternalInput").ap()
    bvb = nc.dram_tensor("bvb", [128, 128], F32, kind="ExternalInput").ap()
    out = nc.dram_tensor("out", [S, D], F32, kind="ExternalOutput").ap()

    with tile.TileContext(nc) as tc:
        with (
            tc.tile_pool(name="consts", bufs=1) as consts,
            tc.tile_pool(name="persist", bufs=1) as persist,
            tc.tile_pool(name="pt", bufs=3) as pt_pool,
            tc.tile_pool(name="aot", bufs=2) as aot_pool,
            tc.tile_pool(name="osb", bufs=4) as osb_pool,
            tc.tile_pool(name="posb", bufs=2) as posb_pool,
            tc.tile_pool(name="small", bufs=4) as small_pool,
            # PSUM (8 banks): sc 2x2, po 1x2, acc 2x1
            tc.tile_pool(name="ps_sc", bufs=2, space="PSUM") as sc_pool,
            tc.tile_pool(name="ps_po", bufs=1, space="PSUM") as po_pool,
            tc.tile_pool(name="ps_acc", bufs=2, space="PSUM") as acc_pool,
        ):
            # ---- constants ----
            ones64f = consts.tile([1, HD], F32)
            nc.vector.memset(ones64f, 1.0)
            ones64 = consts.tile([1, HD], FP16)
            nc.vector.tensor_copy(ones64, ones64f)

            # ---- persistent activations ----
            xtks = [
                persist.tile([128, IC, SEG], BF16, name=f"xtk{s}")
                for s in range(NSEG)
            ]
            kt = persist.tile([128, S], BF16)  # K^T pair [dv, t]
            qt = persist.tile([128, S], BF16)  # Q^T pair
            # V_aug fp8: flat dim = (cp, j, h); 80-padded rows, col 64 = ones
            v2 = persist.tile([128, KCH * 2, 80], FP8)
            nc.vector.memset(v2[:, :, 64:65], 1.0)
            v2r = v2.rearrange("p (cp j h) m -> p cp j h m", cp=NCP, j=2, h=2)

            # ---- DMAs (x^T pre-transposed on host; plain loads) ----
            xtd = xt.rearrange("(c p) t -> p c t", p=128)
            nc.sync.dma_start(xtks[0][:, :, 0:512], xtd[:, :, 0:512])
            wq_sb = consts.tile([128, IC, 128], BF16)
            nc.sync.dma_start(wq_sb, wqt.rearrange("(c p) o -> p c o", p=128))
            wk_sb = consts.tile([128, IC, 128], BF16)
            nc.sync.dma_start(wk_sb, wkt.rearrange("(c p) o -> p c o", p=128))
            bq_sb = consts.tile([128, 1], F32)
            nc.sync.dma_start(bq_sb, bqs)
            bk_sb = consts.tile([128, 1], F32)
            nc.sync.dma_start(bk_sb, bks)
            wv_sb = consts.tile([128, IC, 128], BF16)
            nc.sync.dma_start(wv_sb, wvt.rearrange("(c p) o -> p c o", p=128))
            bvb_sb = consts.tile([128, 128], F32)
            nc.sync.dma_start(bvb_sb, bvb)
            nc.sync.dma_start(xtks[0][:, :, 512:SEG], xtd[:, :, 512:SEG])
            wo_sb = consts.tile([128, D], BF16)
            nc.sync.dma_start(wo_sb, wos)
            for s in range(1, NSEG):
                nc.sync.dma_start(xtks[s], xtd[:, :, s * SEG : (s + 1) * SEG])

            # ---- projection units ----
            def q_unit(tt):
                ps = acc_pool.tile([128, QT], F32, tag="acc", name=f"q{tt}")
                s, ss = divmod(tt, 2)
                for i in range(IC):
                    nc.tensor.matmul(
                        ps,
                        wq_sb[:, i, :],
                        xtks[s][:, i, ss * QT : (ss + 1) * QT],
                        start=(i == 0),
                        stop=(i == IC - 1),
                    )
                nc.scalar.add(qt[:, tt * QT : (tt + 1) * QT], ps, bq_sb[:, 0:1])

            def k_unit(tt, lo=0, hi=QT):
                ps = acc_pool.tile(
                    [128, hi - lo], F32, tag="acc", name=f"k{tt}_{lo}"
                )
                s, ss = divmod(tt, 2)
                for i in range(IC):
                    nc.tensor.matmul(
                        ps,
                        wk_sb[:, i, :],
                        xtks[s][:, i, ss * QT + lo : ss * QT + hi],
                        start=(i == 0),
                        stop=(i == IC - 1),
                    )
                nc.vector.tensor_scalar_add(
                    kt[:, tt * QT + lo : tt * QT + hi], ps, bk_sb[:, 0:1]
                )

            def v_unit(j):
                # V rows for t-chunk j, both heads: [128 t, 128 dv] + bias
                ps = acc_pool.tile([128, 128], F32, tag="acc", name=f"v{j}")
                s, jj = divmod(j, 8)
                for i in range(IC):
                    nc.tensor.matmul(
                        ps,
                        xtks[s][:, i, jj * 128 : (jj + 1) * 128],
                        wv_sb[:, i, :],
                        start=(i == 0),
                        stop=(i == IC - 1),
                    )
                cp, pj = divmod(j, 2)
                nc.vector.tensor_add(
                    v2r[:, cp, pj, :, 0:64],
                    ps.rearrange("p (h d) -> p h d", h=2),
                    bvb_sb.rearrange("p (h d) -> p h d", h=2),
                )

            # upfront: enough for attention (qi=0, pair 0) to start
            q_unit(0)
            k_unit(0, 0, 128)
            v_unit(0)
            k_unit(0, 128, QT)
            v_unit(1)
            k_unit(1)
            v_unit(2)
            v_unit(3)
            pending = []
            for u in (4, 5, 6, 7):
                pending.append(lambda j=u: v_unit(j))
            pending.append(lambda: q_unit(1))
            for tt in range(2, 8):  # k segs with their v chunks
                pending.append(lambda tt=tt: k_unit(tt))
                for j in range(4 * tt, 4 * tt + 4):
                    pending.append(lambda j=j: v_unit(j))
            for tt in range(2, 8):
                pending.append(lambda tt=tt: q_unit(tt))

            pending_slow = []

            aots = {}

            def norm_unit(qi, recs):
                pb2 = acc_pool.tile([128, QT], F32, tag="acc", name=f"pb{qi}")
                nc.tensor.matmul(
                    pb2[0:HD, :], ones64, recs[:, 0:QT], start=True, stop=True
                )
                nc.tensor.matmul(
                    pb2[HD:128, :], ones64, recs[:, QT : 2 * QT],
                    start=True, stop=True,
                )
                nc.vector.scalar_tensor_tensor(
                    aots[qi], pb2, 1.0, posbs[qi], op0=MUL, op1=MUL
                )

            def fin_unit(qi, t4):
                ps = acc_pool.tile([128, D], F32, tag="acc", name=f"f{qi}_{t4}")
                nc.tensor.matmul(
                    ps,
                    aots[qi][:, t4 * 128 : (t4 + 1) * 128],
                    wo_sb,
                    start=True,
                    stop=True,
                )
                osb = osb_pool.tile([128, D], F32, tag="osb")
                if t4 % 2 == 0:
                    nc.scalar.copy(osb, ps)
                else:
                    nc.vector.tensor_copy(osb, ps)
                t0 = qi * QT + t4 * 128
                nc.sync.dma_start(out[t0 : t0 + 128, :], osb)

            posbs = {}

            # ---- attention ----
            for qi in range(NQT):
                qs = qi * QT
                aots[qi] = aot_pool.tile(
                    [128, QT], BF16, tag="aot", name=f"aot{qi}"
                )
                # po: both heads + denominators in one 2-bank psum tile
                po = po_pool.tile([65, 2 * QT], F32, tag="po", name=f"po{qi}")

                def scores(k, qs=qs):
                    pss = sc_pool.tile([128, 2, QT], F32, tag="sc")
                    for hh in range(2):
                        off = hh * HD
                        nc.tensor.matmul(
                            pss[:, hh, :],
                            kt[off : off + HD, k * 128 : (k + 1) * 128],
                            qt[off : off + HD, qs : qs + QT],
                            start=True,
                            stop=True,
                        )
                    return pss

                ptiles = {}

                def exp(k, pss):
                    cp, j = divmod(k, 2)
                    if j == 0:
                        ptiles[cp] = pt_pool.tile(
                            [128, 2, 2, QT], FP8, tag="pt", name=f"pt{cp}"
                        )
                    pt = ptiles[cp]
                    if k % 2 == 0 or k == KCH - 1:
                        # ACT: exact exp, fp8e4 convert
                        nc.scalar.activation(
                            pt[:, :, j, :], pss,
                            mybir.ActivationFunctionType.Exp, scale=0.125,
                        )
                    else:
                        # DVE: e4m3 bit-pattern exp (Schraudolph)
                        nc.vector.tensor_scalar(
                            pt[:, :, j, :].bitcast(U8), pss,
                            LOG2E, SCHRAU_C, op0=MUL, op1=ADD,
                        )

                def attn_v(cp, po=po):
                    pt = ptiles.pop(cp)
                    for hh in range(2):
                        nc.tensor.matmul(
                            po[:, hh * QT : (hh + 1) * QT],
                            v2r[:, cp, :, hh, 0:65],
                            pt[:, hh, :, :],
                            start=(cp == 0),
                            stop=(cp == NCP - 1),
                            perf_mode=DR,
                        )

                # software pipeline: scores/exp run ahead of attn@V
                pss = scores(0)
                exp(0, pss)
                pss = scores(1)
                exp(1, pss)
                for cp in range(NCP):
                    it = qi * NCP + cp
                    # drain deferred projection / norm / fin work FIRST so
                    # their engine-queue entries precede the scores/attn@V
                    # that consume them (k_unit(tt) must precede the
                    # lookahead scores(4tt) below; v_unit(j) must precede
                    # attn_v(j//2)).
                    if it >= 1 and pending:
                        pending.pop(0)()
                        if pending:
                            pending.pop(0)()
                        if pending and len(pending) > 8:
                            pending.pop(0)()
                    elif not pending and pending_slow and it % 2 == 1:
                        pending_slow.pop(0)()
                    for k in (2 * cp + 2, 2 * cp + 3):
                        if k < KCH:
                            pss = scores(k)
                            exp(k, pss)
                    attn_v(cp)

                # free po fast: pull out denominators + numerators, then
                # defer the reciprocal->normalize->fin chain.
                db = small_pool.tile([1, 2 * QT], F32, tag="db")
                nc.scalar.copy(db, po[64:65, :])
                posbN = posb_pool.tile(
                    [128, QT], F32, tag="posb", name=f"posb{qi}"
                )
                last = qi == NQT - 1
                nc.vector.tensor_copy(posbN[0:HD, :], po[0:HD, 0:QT])
                if last:
                    nc.scalar.copy(posbN[HD:128, :], po[0:HD, QT : 2 * QT])
                else:
                    nc.vector.tensor_copy(
                        posbN[HD:128, :], po[0:HD, QT : 2 * QT]
                    )
                posbs[qi] = posbN
                recf = small_pool.tile([1, 2 * QT], F32, tag="recf")
                nc.vector.reciprocal_approx_fast(recf, db)
                recs = small_pool.tile([1, 2 * QT], FP16, tag="rec")
                if last:
                    nc.scalar.copy(recs, recf)
                else:
                    nc.vector.tensor_copy(recs, recf)
                pending_slow.append(lambda qi=qi, recs=recs: norm_unit(qi, recs))
                pending_slow.extend(
                    lambda qi=qi, t4=t4: fin_unit(qi, t4) for t4 in range(4)
                )

            for u in pending + pending_slow:
                u()

    nc.compile()
    return nc


_NC_CACHE = None


def _get_program():
    global _NC_CACHE
    if _NC_CACHE is None:
        _NC_CACHE = _build_program()
    return _NC_CACHE


def prepare_in_maps(x, Wq, bq, Wk, bk, Wv, bv, Wo, bo):
    bf = ml_dtypes.bfloat16
    x = np.ascontiguousarray(np.asarray(x, dtype=np.float32)).astype(bf)
    wqT = np.asarray(Wq, np.float32).T  # [D in, D out-rows]
    wkT = np.asarray(Wk, np.float32).T
    wvT = np.asarray(Wv, np.float32).T
    woT = np.asarray(Wo, np.float32).T  # [D dv, D out]
    bq = np.asarray(bq, np.float32)
    bk = np.asarray(bk, np.float32)
    bv = np.asarray(bv, np.float32)
    in_maps = []
    for core in range(NCORES):
        b = core // PAIRS
        hp = core % PAIRS
        pr = slice(hp * 128, (hp + 1) * 128)
        m = {
            "xt": np.ascontiguousarray(x[b].T),
            "wqt": np.ascontiguousarray(wqT[:, pr]).astype(bf),
            "wkt": np.ascontiguousarray(wkT[:, pr]).astype(bf),
            "wvt": np.ascontiguousarray(wvT[:, pr]).astype(bf),
            "wos": np.ascontiguousarray(woT[pr, :]).astype(bf),
            "bqs": np.ascontiguousarray(bq[pr].reshape(128, 1)),
            "bks": np.ascontiguousarray(bk[pr].reshape(128, 1)),
            "bvb": np.ascontiguousarray(
                np.broadcast_to(bv[pr][None, :], (128, 128))
            ),
        }
        in_maps.append(m)
    return in_maps


def assemble(results, bo):
    out = np.empty((B, S, D), dtype=np.float32)
    bo = np.asarray(bo, np.float32)
    for b in range(B):
        acc = results[b * PAIRS]["out"].astype(np.float32, copy=True)
        for hp in range(1, PAIRS):
            acc += results[b * PAIRS + hp]["out"]
        out[b] = acc + bo[None, :]
    return out


def kernel(x, Wq, bq, Wk, bk, Wv, bv, Wo, bo):
    in_maps = prepare_in_maps(x, Wq, bq, Wk, bk, Wv, bv, Wo, bo)
    nc = _get_program()
    res = run_bass_kernel_spmd(nc, in_maps, core_ids=list(range(NCORES)))
    return assemble(res.results, bo)


# revision 5
# speedup vs baseline: 1.1376x; 1.0996x over previous
"""Multi-head attention (B=2, S=4096, D=512, H=8) on 8 TRN2 NeuronCores.

Sharding: (batch, head-pair) tensor parallel. Core i handles batch i//4
and heads 2*(i%4), 2*(i%4)+1. Each core computes Q/K/V projections only
for its two heads, full S x S attention for those heads, and a PARTIAL
output projection. The host sums the 4 partials per batch (f32) and adds
bo once -- no device collectives.

Key structure (v2): the softmax exp -- the baseline bottleneck at
256 x 1.09us on ACT -- is split across TWO engines, and attn@V runs in
fp8 DoubleRow (contraction 256 = two 128-k-chunks per matmul):

  1. Transposing host prep: x^T loaded [d, t] in 4 segments.
  2. Q^T/K^T = W x^T bf16 (both heads row-packed); V -> fp8e4 V_aug
     [t, 2-chunk-pair, 65] with a ones column (softmax denominator).
  3. Per (q-tile 512, k-chunk 128): 2 row-tiled score matmuls (c=64,
     heads at PE rows 0-63/64-127 run concurrently) -> psum [128,2,512].
     exp: EVEN chunks on ACT (exact exp -> fp8e4 convert); ODD chunks on
     DVE as a single tensor_scalar (x*log2e + 56.05 -> uint8) that
     constructs the e4m3 BIT PATTERN directly (Schraudolph in fp8
     space; constant tuned so the path is bias-free vs exact exp).
  4. attn@V: per (chunk-PAIR, head) one DoubleRow fp8 matmul
     lhsT=[128,2,65] V_aug-pair, rhs=[128,2,512] p-pair, accumulating
     po [65, 1024] (both heads in one 2-bank psum tile; row 64 = the
     softmax denominators).
  5. Normalize per q-tile: denominators [1,1024] -> fast reciprocal ->
     fp16 -> two rank-1 broadcast matmuls -> one scalar_tensor_tensor
     multiply -> aot bf16. Output projection per 128-t chunk, f32 out.
     bo is added on the host during the gather.

Steady state: PE ~scores 512cyc + attnV-DR ~640cyc per chunk-pair;
ACT ~1.09us/even-chunk exp; DVE ~1.19us/odd-chunk exp; aux ops spread
across ACT/DVE for balance.
"""

import numpy as np
import ml_dtypes

import concourse.bass as bass
import concourse.tile as tile
from concourse import bacc, mybir
from concourse.bass_utils import run_bass_kernel_spmd

F32 = mybir.dt.float32
FP16 = mybir.dt.float16
BF16 = mybir.dt.bfloat16
FP8 = mybir.dt.float8e4
U8 = mybir.dt.uint8
MUL = mybir.AluOpType.mult
ADD = mybir.AluOpType.add
DR = mybir.MatmulPerfMode.DoubleRow

B, S, D, H = 2, 4096, 512, 8
HD = D // H  # 64
NCORES = 8
PAIRS = 4  # head-pairs; one per core (per batch)
IC = D // 128  # 4 contraction chunks over d_model
QT = 512  # q tile
NQT = S // QT  # 8
KCH = S // 128  # 32 k chunks
NCP = KCH // 2  # 16 chunk pairs (DoubleRow contracts 2 chunks)
SEG = 1024  # t-columns per transposed DMA segment
NSEG = S // SEG  # 4

LOG2E = 1.4426950408889634
# e4m3 Schraudolph bias: 56 (exponent bias*8) + sawtooth centering +0.5
# for the truncating f32->uint8 convert. Tuned numerically for zero
# multiplicative bias vs the exact-exp path (see session notes).
SCHRAU_C = 56.05


def _build_program():
    nc = bacc.Bacc(
        "TRN2",
        target_bir_lowering=False,
        debug=False,
        enable_asserts=False,
        num_devices=NCORES,
    )
    xt = nc.dram_tensor("xt", [D, S], BF16, kind="ExternalInput").ap()
    wqt = nc.dram_tensor("wqt", [D, 128], BF16, kind="ExternalInput").ap()
    wkt = nc.dram_tensor("wkt", [D, 128], BF16, kind="ExternalInput").ap()
    wvt = nc.dram_tensor("wvt", [D, 128], BF16, kind="ExternalInput").ap()
    wos = nc.dram_tensor("wos", [128, D], BF16, kind="ExternalInput").ap()
    bqs = nc.dram_tensor("bqs", [128, 1], F32, kind="ExternalInput").ap()
    bks = nc.dram_tensor("bks", [128, 1], F32, kind="ExternalInput").ap()
    bvb = nc.dram_tensor("bvb", [128, 128], F32, kind="ExternalInput").ap()
    out = nc.dram_tensor("out", [S, D], F32, kind="ExternalOutput").ap()

    with tile.TileContext(nc) as tc:
        with (
            tc.tile_pool(name="consts", bufs=1) as consts,
            tc.tile_pool(name="persist", bufs=1) as persist,
            tc.tile_pool(name="pt", bufs=3) as pt_pool,
            tc.tile_pool(name="aot", bufs=2) as aot_pool,
            tc.tile_pool(name="osb", bufs=4) as osb_pool,
            tc.tile_pool(name="posb", bufs=2) as posb_pool,
            tc.tile_pool(name="small", bufs=4) as small_pool,
            # PSUM (8 banks): one shared 3x2-bank rotation for scores +
            # proj/fin/pb2 accumulators (breaks the exp->scores WAR chain
            # that a 2-buffer scores pool serializes on), po 1x2 banks.
            tc.tile_pool(name="ps", bufs=3, space="PSUM") as ps_pool,
            tc.tile_pool(name="ps_po", bufs=1, space="PSUM") as po_pool,
        ):
            # ---- constants ----
            ones64f = consts.tile([1, HD], F32)
            nc.vector.memset(ones64f, 1.0)
            ones64 = consts.tile([1, HD], FP16)
            nc.vector.tensor_copy(ones64, ones64f)

            # ---- persistent activations ----
            xtks = [
                persist.tile([128, IC, SEG], BF16, name=f"xtk{s}")
                for s in range(NSEG)
            ]
            kt = persist.tile([128, S], BF16)  # K^T pair [dv, t]
            qt = persist.tile([128, S], BF16)  # Q^T pair
            # V_aug fp8: flat dim = (cp, j, h); 80-padded rows, col 64 = ones
            v2 = persist.tile([128, KCH * 2, 80], FP8)
            nc.vector.memset(v2[:, :, 64:65], 1.0)
            v2r = v2.rearrange("p (cp j h) m -> p cp j h m", cp=NCP, j=2, h=2)

            # ---- DMAs (x^T pre-transposed on host; plain loads) ----
            xtd = xt.rearrange("(c p) t -> p c t", p=128)
            nc.sync.dma_start(xtks[0][:, :, 0:512], xtd[:, :, 0:512])
            wq_sb = consts.tile([128, IC, 128], BF16)
            nc.sync.dma_start(wq_sb, wqt.rearrange("(c p) o -> p c o", p=128))
            wk_sb = consts.tile([128, IC, 128], BF16)
            nc.sync.dma_start(wk_sb, wkt.rearrange("(c p) o -> p c o", p=128))
            bq_sb = consts.tile([128, 1], F32)
            nc.sync.dma_start(bq_sb, bqs)
            bk_sb = consts.tile([128, 1], F32)
            nc.sync.dma_start(bk_sb, bks)
            wv_sb = consts.tile([128, IC, 128], BF16)
            nc.sync.dma_start(wv_sb, wvt.rearrange("(c p) o -> p c o", p=128))
            bvb_sb = consts.tile([128, 128], F32)
            nc.sync.dma_start(bvb_sb, bvb)
            nc.sync.dma_start(xtks[0][:, :, 512:SEG], xtd[:, :, 512:SEG])
            wo_sb = consts.tile([128, D], BF16)
            nc.sync.dma_start(wo_sb, wos)
            for s in range(1, NSEG):
                nc.sync.dma_start(xtks[s], xtd[:, :, s * SEG : (s + 1) * SEG])

            # ---- projection units ----
            def q_unit(tt):
                ps = ps_pool.tile([128, QT], F32, tag="sc", name=f"q{tt}")
                s, ss = divmod(tt, 2)
                for i in range(IC):
                    nc.tensor.matmul(
                        ps,
                        wq_sb[:, i, :],
                        xtks[s][:, i, ss * QT : (ss + 1) * QT],
                        start=(i == 0),
                        stop=(i == IC - 1),
                    )
                nc.scalar.add(qt[:, tt * QT : (tt + 1) * QT], ps, bq_sb[:, 0:1])

            def k_unit(tt, lo=0, hi=QT):
                ps = ps_pool.tile(
                    [128, hi - lo], F32, tag="sc", name=f"k{tt}_{lo}"
                )
                s, ss = divmod(tt, 2)
                for i in range(IC):
                    nc.tensor.matmul(
                        ps,
                        wk_sb[:, i, :],
                        xtks[s][:, i, ss * QT + lo : ss * QT + hi],
                        start=(i == 0),
                        stop=(i == IC - 1),
                    )
                nc.vector.tensor_scalar_add(
                    kt[:, tt * QT + lo : tt * QT + hi], ps, bk_sb[:, 0:1]
                )

            def v_unit(j):
                # V rows for t-chunk j, both heads: [128 t, 128 dv] + bias
                ps = ps_pool.tile([128, 128], F32, tag="sc", name=f"v{j}")
                s, jj = divmod(j, 8)
                for i in range(IC):
                    nc.tensor.matmul(
                        ps,
                        xtks[s][:, i, jj * 128 : (jj + 1) * 128],
                        wv_sb[:, i, :],
                        start=(i == 0),
                        stop=(i == IC - 1),
                    )
                cp, pj = divmod(j, 2)
                nc.vector.tensor_add(
                    v2r[:, cp, pj, :, 0:64],
                    ps.rearrange("p (h d) -> p h d", h=2),
                    bvb_sb.rearrange("p (h d) -> p h d", h=2),
                )

            # upfront: enough for attention (qi=0, pair 0) to start
            q_unit(0)
            k_unit(0, 0, 128)
            v_unit(0)
            k_unit(0, 128, QT)
            v_unit(1)
            k_unit(1)
            v_unit(2)
            v_unit(3)
            pending = []
            for u in (4, 5, 6, 7):
                pending.append(lambda j=u: v_unit(j))
            pending.append(lambda: q_unit(1))
            for tt in range(2, 8):  # k segs with their v chunks
                pending.append(lambda tt=tt: k_unit(tt))
                for j in range(4 * tt, 4 * tt + 4):
                    pending.append(lambda j=j: v_unit(j))
            for tt in range(2, 8):
                pending.append(lambda tt=tt: q_unit(tt))

            pending_slow = []

            aots = {}

            def norm_unit(qi, recs):
                pb2 = ps_pool.tile([128, QT], F32, tag="sc", name=f"pb{qi}")
                nc.tensor.matmul(
                    pb2[0:HD, :], ones64, recs[:, 0:QT], start=True, stop=True
                )
                nc.tensor.matmul(
                    pb2[HD:128, :], ones64, recs[:, QT : 2 * QT],
                    start=True, stop=True,
                )
                nc.vector.scalar_tensor_tensor(
                    aots[qi], pb2, 1.0, posbs[qi], op0=MUL, op1=MUL
                )

            def fin_unit(qi, t4):
                ps = ps_pool.tile([128, D], F32, tag="sc", name=f"f{qi}_{t4}")
                nc.tensor.matmul(
                    ps,
                    aots[qi][:, t4 * 128 : (t4 + 1) * 128],
                    wo_sb,
                    start=True,
                    stop=True,
                )
                osb = osb_pool.tile([128, D], F32, tag="osb")
                if t4 % 2 == 0:
                    nc.scalar.copy(osb, ps)
                else:
                    nc.vector.tensor_copy(osb, ps)
                t0 = qi * QT + t4 * 128
                nc.sync.dma_start(out[t0 : t0 + 128, :], osb)

            posbs = {}

            # ---- attention ----
            for qi in range(NQT):
                qs = qi * QT
                aots[qi] = aot_pool.tile(
                    [128, QT], BF16, tag="aot", name=f"aot{qi}"
                )
                # po: both heads + denominators in one 2-bank psum tile
                po = po_pool.tile([65, 2 * QT], F32, tag="po", name=f"po{qi}")

                def scores(k, qs=qs):
                    pss = ps_pool.tile([128, 2, QT], F32, tag="sc")
                    for hh in range(2):
                        off = hh * HD
                        nc.tensor.matmul(
                            pss[:, hh, :],
                            kt[off : off + HD, k * 128 : (k + 1) * 128],
                            qt[off : off + HD, qs : qs + QT],
                            start=True,
                            stop=True,
                        )
                    return pss

                ptiles = {}

                def exp(k, pss):
                    cp, j = divmod(k, 2)
                    if j == 0:
                        ptiles[cp] = pt_pool.tile(
                            [128, 2, 2, QT], FP8, tag="pt", name=f"pt{cp}"
                        )
                    pt = ptiles[cp]
                    if k % 2 == 0 or k == KCH - 1:
                        # ACT: exact exp, fp8e4 convert
                        nc.scalar.activation(
                            pt[:, :, j, :], pss,
                            mybir.ActivationFunctionType.Exp, scale=0.125,
                        )
                    else:
                        # DVE: e4m3 bit-pattern exp (Schraudolph)
                        nc.vector.tensor_scalar(
                            pt[:, :, j, :].bitcast(U8), pss,
                            LOG2E, SCHRAU_C, op0=MUL, op1=ADD,
                        )

                def attn_v(cp, po=po):
                    pt = ptiles.pop(cp)
                    for hh in range(2):
                        nc.tensor.matmul(
                            po[:, hh * QT : (hh + 1) * QT],
                            v2r[:, cp, :, hh, 0:65],
                            pt[:, hh, :, :],
                            start=(cp == 0),
                            stop=(cp == NCP - 1),
                            perf_mode=DR,
                        )

                # software pipeline: scores/exp run ahead of attn@V
                pss = scores(0)
                exp(0, pss)
                pss = scores(1)
                exp(1, pss)
                for cp in range(NCP):
                    it = qi * NCP + cp
                    # drain deferred projection / norm / fin work FIRST so
                    # their engine-queue entries precede the scores/attn@V
                    # that consume them (k_unit(tt) must precede the
                    # lookahead scores(4tt) below; v_unit(j) must precede
                    # attn_v(j//2)).
                    if it >= 1 and pending:
                        pending.pop(0)()
                        if pending:
                            pending.pop(0)()
                        if pending and len(pending) > 8:
                            pending.pop(0)()
                    elif not pending and pending_slow and it % 2 == 1:
                        pending_slow.pop(0)()
                    for k in (2 * cp + 2, 2 * cp + 3):
                        if k < KCH:
                            pss = scores(k)
                            exp(k, pss)
                    attn_v(cp)

                # free po fast: pull out denominators + numerators, then
                # defer the reciprocal->normalize->fin chain.
                db = small_pool.tile([1, 2 * QT], F32, tag="db")
                nc.scalar.copy(db, po[64:65, :])
                posbN = posb_pool.tile(
                    [128, QT], F32, tag="posb", name=f"posb{qi}"
                )
                last = qi == NQT - 1
                nc.vector.tensor_copy(posbN[0:HD, :], po[0:HD, 0:QT])
                if last:
                    nc.scalar.copy(posbN[HD:128, :], po[0:HD, QT : 2 * QT])
                else:
                    nc.vector.tensor_copy(
                        posbN[HD:128, :], po[0:HD, QT : 2 * QT]
                    )
                posbs[qi] = posbN
                recf = small_pool.tile([1, 2 * QT], F32, tag="recf")
                nc.vector.reciprocal_approx_fast(recf, db)
                recs = small_pool.tile([1, 2 * QT], FP16, tag="rec")
                if last:
                    nc.scalar.copy(recs, recf)
                else:
                    nc.vector.tensor_copy(recs, recf)
                pending_slow.append(lambda qi=qi, recs=recs: norm_unit(qi, recs))
                pending_slow.extend(
                    lambda qi=qi, t4=t4: fin_unit(qi, t4) for t4 in range(4)
                )

            for u in pending + pending_slow:
                u()

    nc.compile()
    return nc


_NC_CACHE = None


def _get_program():
    global _NC_CACHE
    if _NC_CACHE is None:
        _NC_CACHE = _build_program()
    return _NC_CACHE


def prepare_in_maps(x, Wq, bq, Wk, bk, Wv, bv, Wo, bo):
    bf = ml_dtypes.bfloat16
    x = np.ascontiguousarray(np.asarray(x, dtype=np.float32)).astype(bf)
    wqT = np.asarray(Wq, np.float32).T  # [D in, D out-rows]
    wkT = np.asarray(Wk, np.float32).T
    wvT = np.asarray(Wv, np.float32).T
    woT = np.asarray(Wo, np.float32).T  # [D dv, D out]
    bq = np.asarray(bq, np.float32)
    bk = np.asarray(bk, np.float32)
    bv = np.asarray(bv, np.float32)
    in_maps = []
    for core in range(NCORES):
        b = core // PAIRS
        hp = core % PAIRS
        pr = slice(hp * 128, (hp + 1) * 128)
        m = {
            "xt": np.ascontiguousarray(x[b].T),
            "wqt": np.ascontiguousarray(wqT[:, pr]).astype(bf),
            "wkt": np.ascontiguousarray(wkT[:, pr]).astype(bf),
            "wvt": np.ascontiguousarray(wvT[:, pr]).astype(bf),
            "wos": np.ascontiguousarray(woT[pr, :]).astype(bf),
            "bqs": np.ascontiguousarray(bq[pr].reshape(128, 1)),
            "bks": np.ascontiguousarray(bk[pr].reshape(128, 1)),
            "bvb": np.ascontiguousarray(
                np.broadcast_to(bv[pr][None, :], (128, 128))
            ),
        }
        in_maps.append(m)
    return in_maps


def assemble(results, bo):
    out = np.empty((B, S, D), dtype=np.float32)
    bo = np.asarray(bo, np.float32)
    for b in range(B):
        acc = results[b * PAIRS]["out"].astype(np.float32, copy=True)
        for hp in range(1, PAIRS):
            acc += results[b * PAIRS + hp]["out"]
        out[b] = acc + bo[None, :]
    return out


def kernel(x, Wq, bq, Wk, bk, Wv, bv, Wo, bo):
    in_maps = prepare_in_maps(x, Wq, bq, Wk, bk, Wv, bv, Wo, bo)
    nc = _get_program()
    res = run_bass_kernel_spmd(nc, in_maps, core_ids=list(range(NCORES)))
    return assemble(res.results, bo)


# revision 7
# speedup vs baseline: 1.2082x; 1.0620x over previous
"""Multi-head attention (B=2, S=4096, D=512, H=8) on 8 TRN2 NeuronCores.

Sharding: (batch, head-pair) tensor parallel. Core i handles batch i//4
and heads 2*(i%4), 2*(i%4)+1. Each core computes Q/K/V projections only
for its two heads, full S x S attention for those heads, and a PARTIAL
output projection. The host sums the 4 partials per batch (f32) and adds
bo once -- no device collectives.

Key structure (v2): the softmax exp -- the baseline bottleneck at
256 x 1.09us on ACT -- is split across TWO engines, and attn@V runs in
fp8 DoubleRow (contraction 256 = two 128-k-chunks per matmul):

  1. Transposing host prep: x^T loaded [d, t] in 4 segments.
  2. Q^T/K^T = W x^T bf16 (both heads row-packed); V -> fp8e4 V_aug
     [t, 2-chunk-pair, 65] with a ones column (softmax denominator).
  3. Per (q-tile 512, k-chunk 128): 2 row-tiled score matmuls (c=64,
     heads at PE rows 0-63/64-127 run concurrently) -> psum [128,2,512].
     exp: EVEN chunks on ACT (exact exp -> fp8e4 convert); ODD chunks on
     DVE as a single tensor_scalar (x*log2e + 56.05 -> uint8) that
     constructs the e4m3 BIT PATTERN directly (Schraudolph in fp8
     space; constant tuned so the path is bias-free vs exact exp).
  4. attn@V: per (chunk-PAIR, head) one DoubleRow fp8 matmul
     lhsT=[128,2,65] V_aug-pair, rhs=[128,2,512] p-pair, accumulating
     po [65, 1024] (both heads in one 2-bank psum tile; row 64 = the
     softmax denominators).
  5. Normalize per q-tile: denominators [1,1024] -> fast reciprocal ->
     fp16 -> two rank-1 broadcast matmuls -> one scalar_tensor_tensor
     multiply -> aot bf16. Output projection per 128-t chunk, f32 out.
     bo is added on the host during the gather.

Steady state: PE ~scores 512cyc + attnV-DR ~640cyc per chunk-pair;
ACT ~1.09us/even-chunk exp; DVE ~1.19us/odd-chunk exp; aux ops spread
across ACT/DVE for balance.
"""

import numpy as np
import ml_dtypes

import concourse.bass as bass
import concourse.tile as tile
from concourse import bacc, mybir
from concourse.bass_utils import run_bass_kernel_spmd

F32 = mybir.dt.float32
FP16 = mybir.dt.float16
F32R = mybir.dt.float32r
BF16 = mybir.dt.bfloat16
FP8 = mybir.dt.float8e4
U8 = mybir.dt.uint8
MUL = mybir.AluOpType.mult
ADD = mybir.AluOpType.add
DR = mybir.MatmulPerfMode.DoubleRow

B, S, D, H = 2, 4096, 512, 8
HD = D // H  # 64
NCORES = 8
PAIRS = 4  # head-pairs; one per core (per batch)
IC = D // 128  # 4 contraction chunks over d_model
QT = 512  # q tile
NQT = S // QT  # 8
KCH = S // 128  # 32 k chunks
NCP = KCH // 2  # 16 chunk pairs (DoubleRow contracts 2 chunks)
SEG = 1024  # t-columns per transposed DMA segment
NSEG = S // SEG  # 4

LOG2E = 1.4426950408889634
# e4m3 Schraudolph bias: 56 (exponent bias*8) + sawtooth centering +0.5
# for the truncating f32->uint8 convert. Tuned numerically for zero
# multiplicative bias vs the exact-exp path (see session notes).
SCHRAU_C = 56.05


def _build_program():
    nc = bacc.Bacc(
        "TRN2",
        target_bir_lowering=False,
        debug=False,
        enable_asserts=False,
        num_devices=NCORES,
    )
    xt = nc.dram_tensor("xt", [D, S], BF16, kind="ExternalInput").ap()
    wqt = nc.dram_tensor("wqt", [D, 128], BF16, kind="ExternalInput").ap()
    wkt = nc.dram_tensor("wkt", [D, 128], BF16, kind="ExternalInput").ap()
    wvt = nc.dram_tensor("wvt", [D, 128], BF16, kind="ExternalInput").ap()
    wos = nc.dram_tensor("wos", [128, D], BF16, kind="ExternalInput").ap()
    bqs = nc.dram_tensor("bqs", [128, 1], F32, kind="ExternalInput").ap()
    bks = nc.dram_tensor("bks", [128, 1], F32, kind="ExternalInput").ap()
    bvb = nc.dram_tensor("bvb", [128, 128], F32, kind="ExternalInput").ap()
    out = nc.dram_tensor("out", [S, D], F32, kind="ExternalOutput").ap()

    with tile.TileContext(nc) as tc:
        with (
            tc.tile_pool(name="consts", bufs=1) as consts,
            tc.tile_pool(name="persist", bufs=1) as persist,
            tc.tile_pool(name="pt", bufs=3) as pt_pool,
            tc.tile_pool(name="aot", bufs=2) as aot_pool,
            tc.tile_pool(name="osb", bufs=4) as osb_pool,
            tc.tile_pool(name="posb", bufs=2) as posb_pool,
            tc.tile_pool(name="small", bufs=4) as small_pool,
            # PSUM (8 banks): one shared 3x2-bank rotation for scores +
            # proj/fin/pb2 accumulators (breaks the exp->scores WAR chain
            # that a 2-buffer scores pool serializes on), po 1x2 banks.
            tc.tile_pool(name="ps", bufs=3, space="PSUM") as ps_pool,
            tc.tile_pool(name="ps_po", bufs=1, space="PSUM") as po_pool,
        ):
            # ---- constants ----
            ones64f = consts.tile([1, HD], F32)
            nc.vector.memset(ones64f, 1.0)
            ones64 = consts.tile([1, HD], FP16)
            nc.vector.tensor_copy(ones64, ones64f)

            # ---- persistent activations ----
            xtks = [
                persist.tile([128, IC, SEG], BF16, name=f"xtk{s}")
                for s in range(NSEG)
            ]
            kt = persist.tile([128, S], BF16)  # K^T pair [dv, t]
            qt = persist.tile([128, S], BF16)  # Q^T pair
            # V_aug fp8: flat dim = (cp, j, h); 80-padded rows, col 64 = ones
            v2 = persist.tile([128, KCH * 2, 80], FP8)
            nc.vector.memset(v2[:, :, 64:65], 1.0)
            v2r = v2.rearrange("p (cp j h) m -> p cp j h m", cp=NCP, j=2, h=2)

            # ---- DMAs (x^T pre-transposed on host; plain loads) ----
            xtd = xt.rearrange("(c p) t -> p c t", p=128)
            nc.sync.dma_start(xtks[0][:, :, 0:512], xtd[:, :, 0:512])
            wq_sb = consts.tile([128, IC, 128], BF16)
            nc.sync.dma_start(wq_sb, wqt.rearrange("(c p) o -> p c o", p=128))
            wk_sb = consts.tile([128, IC, 128], BF16)
            nc.sync.dma_start(wk_sb, wkt.rearrange("(c p) o -> p c o", p=128))
            bq_sb = consts.tile([128, 1], F32)
            nc.sync.dma_start(bq_sb, bqs)
            bk_sb = consts.tile([128, 1], F32)
            nc.sync.dma_start(bk_sb, bks)
            wv_sb = consts.tile([128, IC, 128], BF16)
            nc.sync.dma_start(wv_sb, wvt.rearrange("(c p) o -> p c o", p=128))
            bvb_sb = consts.tile([128, 128], F32)
            nc.sync.dma_start(bvb_sb, bvb)
            nc.sync.dma_start(xtks[0][:, :, 512:SEG], xtd[:, :, 512:SEG])
            wo_sb = consts.tile([128, D], BF16)
            nc.sync.dma_start(wo_sb, wos)
            for s in range(1, NSEG):
                nc.sync.dma_start(xtks[s], xtd[:, :, s * SEG : (s + 1) * SEG])

            # ---- projection units ----
            def q_unit(tt):
                ps = ps_pool.tile([128, QT], F32, tag="sc", name=f"q{tt}")
                s, ss = divmod(tt, 2)
                for i in range(IC):
                    nc.tensor.matmul(
                        ps,
                        wq_sb[:, i, :],
                        xtks[s][:, i, ss * QT : (ss + 1) * QT],
                        start=(i == 0),
                        stop=(i == IC - 1),
                    )
                nc.scalar.add(qt[:, tt * QT : (tt + 1) * QT], ps, bq_sb[:, 0:1])

            def k_unit(tt, lo=0, hi=QT):
                ps = ps_pool.tile(
                    [128, hi - lo], F32, tag="sc", name=f"k{tt}_{lo}"
                )
                s, ss = divmod(tt, 2)
                for i in range(IC):
                    nc.tensor.matmul(
                        ps,
                        wk_sb[:, i, :],
                        xtks[s][:, i, ss * QT + lo : ss * QT + hi],
                        start=(i == 0),
                        stop=(i == IC - 1),
                    )
                nc.vector.tensor_scalar_add(
                    kt[:, tt * QT + lo : tt * QT + hi], ps, bk_sb[:, 0:1]
                )

            def v_unit(j):
                # V rows for t-chunk j, both heads: [128 t, 128 dv] + bias
                ps = ps_pool.tile([128, 128], F32, tag="sc", name=f"v{j}")
                s, jj = divmod(j, 8)
                for i in range(IC):
                    nc.tensor.matmul(
                        ps,
                        xtks[s][:, i, jj * 128 : (jj + 1) * 128],
                        wv_sb[:, i, :],
                        start=(i == 0),
                        stop=(i == IC - 1),
                    )
                cp, pj = divmod(j, 2)
                nc.vector.tensor_add(
                    v2r[:, cp, pj, :, 0:64],
                    ps.rearrange("p (h d) -> p h d", h=2),
                    bvb_sb.rearrange("p (h d) -> p h d", h=2),
                )

            # upfront: enough for attention (qi=0, pair 0) to start
            q_unit(0)
            k_unit(0, 0, 128)
            v_unit(0)
            k_unit(0, 128, QT)
            v_unit(1)
            k_unit(1)
            v_unit(2)
            v_unit(3)
            pending = []
            for u in (4, 5, 6, 7):
                pending.append(lambda j=u: v_unit(j))
            pending.append(lambda: q_unit(1))
            for tt in range(2, 8):  # k segs with their v chunks
                pending.append(lambda tt=tt: k_unit(tt))
                for j in range(4 * tt, 4 * tt + 4):
                    pending.append(lambda j=j: v_unit(j))
            for tt in range(2, 8):
                pending.append(lambda tt=tt: q_unit(tt))

            pending_slow = []

            aots = {}

            def rec_unit(qi, db, recf, recs):
                nc.vector.reciprocal_approx_fast(recf, db)
                nc.vector.tensor_copy(recs, recf)

            def norm_unit(qi, recs):
                pb2 = ps_pool.tile([128, QT], F32, tag="sc", name=f"pb{qi}")
                nc.tensor.matmul(
                    pb2[0:HD, :], ones64, recs[:, 0:QT], start=True, stop=True
                )
                nc.tensor.matmul(
                    pb2[HD:128, :], ones64, recs[:, QT : 2 * QT],
                    start=True, stop=True,
                )
                nc.vector.scalar_tensor_tensor(
                    aots[qi], pb2, 1.0, posbs[qi], op0=MUL, op1=MUL
                )

            def fin_unit(qi, t4):
                ps = ps_pool.tile([128, D], F32, tag="sc", name=f"f{qi}_{t4}")
                nc.tensor.matmul(
                    ps,
                    aots[qi][:, t4 * 128 : (t4 + 1) * 128],
                    wo_sb,
                    start=True,
                    stop=True,
                )
                osb = osb_pool.tile([128, D], F32, tag="osb")
                if t4 % 2 == 0:
                    nc.scalar.copy(osb, ps)
                else:
                    nc.vector.tensor_copy(osb, ps)
                t0 = qi * QT + t4 * 128
                nc.sync.dma_start(out[t0 : t0 + 128, :], osb)

            posbs = {}

            # ---- attention ----
            for qi in range(NQT):
                qs = qi * QT
                aots[qi] = aot_pool.tile(
                    [128, QT], BF16, tag="aot", name=f"aot{qi}"
                )
                # po: both heads + denominators in one 2-bank psum tile
                po = po_pool.tile([65, 2 * QT], F32, tag="po", name=f"po{qi}")

                def scores(k, qs=qs):
                    pss = ps_pool.tile([128, 2, QT], F32, tag="sc")
                    for hh in range(2):
                        off = hh * HD
                        nc.tensor.matmul(
                            pss[:, hh, :],
                            kt[off : off + HD, k * 128 : (k + 1) * 128],
                            qt[off : off + HD, qs : qs + QT],
                            start=True,
                            stop=True,
                        )
                    return pss

                ptiles = {}

                def exp(k, pss):
                    cp, j = divmod(k, 2)
                    if j == 0:
                        ptiles[cp] = pt_pool.tile(
                            [128, 2, 2, QT], FP8, tag="pt", name=f"pt{cp}"
                        )
                    pt = ptiles[cp]
                    if k % 2 == 0 or k >= KCH - 3:
                        # ACT: exact exp, fp8e4 convert
                        nc.scalar.activation(
                            pt[:, :, j, :], pss,
                            mybir.ActivationFunctionType.Exp, scale=0.125,
                        )
                    else:
                        # DVE: e4m3 bit-pattern exp (Schraudolph)
                        nc.vector.tensor_scalar(
                            pt[:, :, j, :].bitcast(U8), pss,
                            LOG2E, SCHRAU_C, op0=MUL, op1=ADD,
                        )

                def attn_v(cp, po=po):
                    pt = ptiles.pop(cp)
                    for hh in range(2):
                        nc.tensor.matmul(
                            po[:, hh * QT : (hh + 1) * QT],
                            v2r[:, cp, :, hh, 0:65],
                            pt[:, hh, :, :],
                            start=(cp == 0),
                            stop=(cp == NCP - 1),
                            perf_mode=DR,
                        )

                # software pipeline: scores/exp run ahead of attn@V
                pss = scores(0)
                exp(0, pss)
                pss = scores(1)
                exp(1, pss)
                for cp in range(NCP):
                    it = qi * NCP + cp
                    # drain deferred projection / norm / fin work FIRST so
                    # their engine-queue entries precede the scores/attn@V
                    # that consume them (k_unit(tt) must precede the
                    # lookahead scores(4tt) below; v_unit(j) must precede
                    # attn_v(j//2)).
                    if it >= 1 and pending:
                        pending.pop(0)()
                        if pending:
                            pending.pop(0)()
                        if pending and len(pending) > 8:
                            pending.pop(0)()
                    elif not pending and pending_slow and it % 2 == 1:
                        pending_slow.pop(0)()
                    for k in (2 * cp + 2, 2 * cp + 3):
                        if k < KCH:
                            pss = scores(k)
                            exp(k, pss)
                    attn_v(cp)

                # free po fast: the four po-readers run two-per-engine in
                # parallel; the reciprocal is deferred off the boundary.
                db = small_pool.tile([1, 2 * QT], F32, tag="db")
                posbN = posb_pool.tile(
                    [128, QT], F32, tag="posb", name=f"posb{qi}"
                )
                nc.scalar.copy(db[:, 0:QT], po[64:65, 0:QT])
                nc.vector.tensor_copy(db[:, QT : 2 * QT], po[64:65, QT : 2 * QT])
                nc.vector.tensor_copy(posbN[0:HD, :], po[0:HD, 0:QT])
                nc.scalar.copy(posbN[HD:128, :], po[0:HD, QT : 2 * QT])
                posbs[qi] = posbN
                recf = small_pool.tile([1, 2 * QT], F32, tag="recf")
                recs = small_pool.tile([1, 2 * QT], FP16, tag="rec")
                last = qi == NQT - 1
                if last:
                    rec_unit(qi, db, recf, recs)
                else:
                    pending_slow.append(
                        lambda qi=qi, db=db, recf=recf, recs=recs: rec_unit(
                            qi, db, recf, recs
                        )
                    )
                pending_slow.append(lambda qi=qi, recs=recs: norm_unit(qi, recs))
                pending_slow.extend(
                    lambda qi=qi, t4=t4: fin_unit(qi, t4) for t4 in range(4)
                )

            for u in pending + pending_slow:
                u()

    nc.compile()
    return nc


_NC_CACHE = None


def _get_program():
    global _NC_CACHE
    if _NC_CACHE is None:
        _NC_CACHE = _build_program()
    return _NC_CACHE


def prepare_in_maps(x, Wq, bq, Wk, bk, Wv, bv, Wo, bo):
    bf = ml_dtypes.bfloat16
    x = np.ascontiguousarray(np.asarray(x, dtype=np.float32)).astype(bf)
    wqT = np.asarray(Wq, np.float32).T  # [D in, D out-rows]
    wkT = np.asarray(Wk, np.float32).T
    wvT = np.asarray(Wv, np.float32).T
    woT = np.asarray(Wo, np.float32).T  # [D dv, D out]
    bq = np.asarray(bq, np.float32)
    bk = np.asarray(bk, np.float32)
    bv = np.asarray(bv, np.float32)
    in_maps = []
    for core in range(NCORES):
        b = core // PAIRS
        hp = core % PAIRS
        pr = slice(hp * 128, (hp + 1) * 128)
        m = {
            "xt": np.ascontiguousarray(x[b].T),
            "wqt": np.ascontiguousarray(wqT[:, pr]).astype(bf),
            "wkt": np.ascontiguousarray(wkT[:, pr]).astype(bf),
            "wvt": np.ascontiguousarray(wvT[:, pr]).astype(bf),
            "wos": np.ascontiguousarray(woT[pr, :]).astype(bf),
            "bqs": np.ascontiguousarray(bq[pr].reshape(128, 1)),
            "bks": np.ascontiguousarray(bk[pr].reshape(128, 1)),
            "bvb": np.ascontiguousarray(
                np.broadcast_to(bv[pr][None, :], (128, 128))
            ),
        }
        in_maps.append(m)
    return in_maps


def assemble(results, bo):
    out = np.empty((B, S, D), dtype=np.float32)
    bo = np.asarray(bo, np.float32)
    for b in range(B):
        acc = results[b * PAIRS]["out"].astype(np.float32, copy=True)
        for hp in range(1, PAIRS):
            acc += results[b * PAIRS + hp]["out"]
        out[b] = acc + bo[None, :]
    return out


def kernel(x, Wq, bq, Wk, bk, Wv, bv, Wo, bo):
    in_maps = prepare_in_maps(x, Wq, bq, Wk, bk, Wv, bv, Wo, bo)
    nc = _get_program()
    res = run_bass_kernel_spmd(nc, in_maps, core_ids=list(range(NCORES)))
    return assemble(res.results, bo)


# revision 11
# speedup vs baseline: 1.2124x; 1.0035x over previous
"""Multi-head attention (B=2, S=4096, D=512, H=8) on 8 TRN2 NeuronCores.

Sharding: (batch, head-pair) tensor parallel. Core i handles batch i//4
and heads 2*(i%4), 2*(i%4)+1. Each core computes Q/K/V projections only
for its two heads, full S x S attention for those heads, and a PARTIAL
output projection. The host sums the 4 partials per batch (f32) and adds
bo once -- no device collectives.

Key structure (v2): the softmax exp -- the baseline bottleneck at
256 x 1.09us on ACT -- is split across TWO engines, and attn@V runs in
fp8 DoubleRow (contraction 256 = two 128-k-chunks per matmul):

  1. Transposing host prep: x^T loaded [d, t] in 4 segments.
  2. Q^T/K^T = W x^T bf16 (both heads row-packed); V -> fp8e4 V_aug
     [t, 2-chunk-pair, 65] with a ones column (softmax denominator).
  3. Per (q-tile 512, k-chunk 128): 2 row-tiled score matmuls (c=64,
     heads at PE rows 0-63/64-127 run concurrently) -> psum [128,2,512].
     exp: EVEN chunks on ACT (exact exp -> fp8e4 convert); ODD chunks on
     DVE as a single tensor_scalar (x*log2e + 56.05 -> uint8) that
     constructs the e4m3 BIT PATTERN directly (Schraudolph in fp8
     space; constant tuned so the path is bias-free vs exact exp).
  4. attn@V: per (chunk-PAIR, head) one DoubleRow fp8 matmul
     lhsT=[128,2,65] V_aug-pair, rhs=[128,2,512] p-pair, accumulating
     po [65, 1024] (both heads in one 2-bank psum tile; row 64 = the
     softmax denominators).
  5. Normalize per q-tile: denominators [1,1024] -> fast reciprocal ->
     fp16 -> two rank-1 broadcast matmuls -> one scalar_tensor_tensor
     multiply -> aot bf16. Output projection per 128-t chunk, f32 out.
     bo is added on the host during the gather.

Steady state: PE ~scores 512cyc + attnV-DR ~640cyc per chunk-pair;
ACT ~1.09us/even-chunk exp; DVE ~1.19us/odd-chunk exp; aux ops spread
across ACT/DVE for balance.
"""

import numpy as np
import ml_dtypes

import concourse.bass as bass
import concourse.tile as tile
from concourse import bacc, mybir
from concourse.bass_utils import run_bass_kernel_spmd

F32 = mybir.dt.float32
FP16 = mybir.dt.float16
F32R = mybir.dt.float32r
BF16 = mybir.dt.bfloat16
FP8 = mybir.dt.float8e4
U8 = mybir.dt.uint8
MUL = mybir.AluOpType.mult
ADD = mybir.AluOpType.add
DR = mybir.MatmulPerfMode.DoubleRow

B, S, D, H = 2, 4096, 512, 8
HD = D // H  # 64
NCORES = 8
PAIRS = 4  # head-pairs; one per core (per batch)
IC = D // 128  # 4 contraction chunks over d_model
QT = 512  # q tile
NQT = S // QT  # 8
KCH = S // 128  # 32 k chunks
NCP = KCH // 2  # 16 chunk pairs (DoubleRow contracts 2 chunks)
SEG = 1024  # t-columns per transposed DMA segment
NSEG = S // SEG  # 4

LOG2E = 1.4426950408889634
# e4m3 Schraudolph bias: 56 (exponent bias*8) + sawtooth centering +0.5
# for the truncating f32->uint8 convert. Tuned numerically for zero
# multiplicative bias vs the exact-exp path (see session notes).
SCHRAU_C = 56.05


def _build_program():
    nc = bacc.Bacc(
        "TRN2",
        target_bir_lowering=False,
        debug=False,
        enable_asserts=False,
        num_devices=NCORES,
    )
    xt = nc.dram_tensor("xt", [D, S], BF16, kind="ExternalInput").ap()
    wqt = nc.dram_tensor("wqt", [D, 128], BF16, kind="ExternalInput").ap()
    wkt = nc.dram_tensor("wkt", [D, 128], BF16, kind="ExternalInput").ap()
    wvt = nc.dram_tensor("wvt", [D, 128], BF16, kind="ExternalInput").ap()
    wos = nc.dram_tensor("wos", [128, D], BF16, kind="ExternalInput").ap()
    bqs = nc.dram_tensor("bqs", [128, 1], F32, kind="ExternalInput").ap()
    bks = nc.dram_tensor("bks", [128, 1], F32, kind="ExternalInput").ap()
    bvb = nc.dram_tensor("bvb", [128, 128], F32, kind="ExternalInput").ap()
    out = nc.dram_tensor("out", [S, D], F32, kind="ExternalOutput").ap()

    with tile.TileContext(nc) as tc:
        with (
            tc.tile_pool(name="consts", bufs=1) as consts,
            tc.tile_pool(name="persist", bufs=1) as persist,
            tc.tile_pool(name="pt", bufs=3) as pt_pool,
            tc.tile_pool(name="aot", bufs=2) as aot_pool,
            tc.tile_pool(name="osb", bufs=4) as osb_pool,
            tc.tile_pool(name="posb", bufs=2) as posb_pool,
            tc.tile_pool(name="small", bufs=4) as small_pool,
            # PSUM (8 banks): one shared 3x2-bank rotation for scores +
            # proj/fin/pb2 accumulators (breaks the exp->scores WAR chain
            # that a 2-buffer scores pool serializes on), po 1x2 banks.
            tc.tile_pool(name="ps", bufs=3, space="PSUM") as ps_pool,
            tc.tile_pool(name="ps_po", bufs=1, space="PSUM") as po_pool,
        ):
            # ---- constants ----
            ones64f = consts.tile([1, HD], F32)
            nc.vector.memset(ones64f, 1.0)
            ones64 = consts.tile([1, HD], FP16)
            nc.vector.tensor_copy(ones64, ones64f)

            # ---- persistent activations ----
            xtks = [
                persist.tile([128, IC, SEG], BF16, name=f"xtk{s}")
                for s in range(NSEG)
            ]
            kt = persist.tile([128, S], BF16)  # K^T pair [dv, t]
            qt = persist.tile([128, S], BF16)  # Q^T pair
            # V_aug fp8: flat dim = (cp, j, h); 80-padded rows, col 64 = ones
            v2 = persist.tile([128, KCH * 2, 80], FP8)
            nc.vector.memset(v2[:, :, 64:65], 1.0)
            v2r = v2.rearrange("p (cp j h) m -> p cp j h m", cp=NCP, j=2, h=2)

            # ---- DMAs (x^T pre-transposed on host; plain loads) ----
            xtd = xt.rearrange("(c p) t -> p c t", p=128)
            wq_sb = consts.tile([128, IC, 128], BF16)
            nc.sync.dma_start(wq_sb, wqt.rearrange("(c p) o -> p c o", p=128))
            bq_sb = consts.tile([128, 1], F32)
            nc.sync.dma_start(bq_sb, bqs)
            wk_sb = consts.tile([128, IC, 128], BF16)
            nc.sync.dma_start(wk_sb, wkt.rearrange("(c p) o -> p c o", p=128))
            bk_sb = consts.tile([128, 1], F32)
            nc.sync.dma_start(bk_sb, bks)
            nc.sync.dma_start(xtks[0][:, :, 0:512], xtd[:, :, 0:512])
            wv_sb = consts.tile([128, IC, 128], BF16)
            nc.sync.dma_start(wv_sb, wvt.rearrange("(c p) o -> p c o", p=128))
            bvb_sb = consts.tile([128, 128], F32)
            nc.sync.dma_start(bvb_sb, bvb)
            nc.sync.dma_start(xtks[0][:, :, 512:SEG], xtd[:, :, 512:SEG])
            wo_sb = consts.tile([128, D], BF16)
            nc.sync.dma_start(wo_sb, wos)
            for s in range(1, NSEG):
                nc.sync.dma_start(xtks[s], xtd[:, :, s * SEG : (s + 1) * SEG])

            # ---- projection units ----
            def q_unit(tt):
                ps = ps_pool.tile([128, QT], F32, tag="sc", name=f"q{tt}")
                s, ss = divmod(tt, 2)
                for i in range(IC):
                    nc.tensor.matmul(
                        ps,
                        wq_sb[:, i, :],
                        xtks[s][:, i, ss * QT : (ss + 1) * QT],
                        start=(i == 0),
                        stop=(i == IC - 1),
                    )
                nc.scalar.add(qt[:, tt * QT : (tt + 1) * QT], ps, bq_sb[:, 0:1])

            def k_unit(tt, lo=0, hi=QT):
                ps = ps_pool.tile(
                    [128, hi - lo], F32, tag="sc", name=f"k{tt}_{lo}"
                )
                s, ss = divmod(tt, 2)
                for i in range(IC):
                    nc.tensor.matmul(
                        ps,
                        wk_sb[:, i, :],
                        xtks[s][:, i, ss * QT + lo : ss * QT + hi],
                        start=(i == 0),
                        stop=(i == IC - 1),
                    )
                nc.vector.tensor_scalar_add(
                    kt[:, tt * QT + lo : tt * QT + hi], ps, bk_sb[:, 0:1]
                )

            def v_unit(j):
                # V rows for t-chunk j, both heads: [128 t, 128 dv] + bias
                ps = ps_pool.tile([128, 128], F32, tag="sc", name=f"v{j}")
                s, jj = divmod(j, 8)
                for i in range(IC):
                    nc.tensor.matmul(
                        ps,
                        xtks[s][:, i, jj * 128 : (jj + 1) * 128],
                        wv_sb[:, i, :],
                        start=(i == 0),
                        stop=(i == IC - 1),
                    )
                cp, pj = divmod(j, 2)
                nc.vector.tensor_add(
                    v2r[:, cp, pj, :, 0:64],
                    ps.rearrange("p (h d) -> p h d", h=2),
                    bvb_sb.rearrange("p (h d) -> p h d", h=2),
                )

            # upfront: bare minimum for scores(0) -- q-tile 0 and the
            # first 128 kt columns. Everything else drains into the
            # attention loop's slack.
            q_unit(0)
            k_unit(0, 0, 128)
            pending = [
                lambda: v_unit(0),
                lambda: v_unit(1),
                lambda: k_unit(1),
                lambda: v_unit(2),
                lambda: v_unit(3),
            ]
            for u in (4, 5, 6, 7):
                pending.append(lambda j=u: v_unit(j))
            pending.append(lambda: q_unit(1))
            for tt in range(2, 8):  # k segs with their v chunks
                pending.append(lambda tt=tt: k_unit(tt))
                for j in range(4 * tt, 4 * tt + 4):
                    pending.append(lambda j=j: v_unit(j))
            for tt in range(2, 8):
                pending.append(lambda tt=tt: q_unit(tt))

            pending_slow = []

            aots = {}

            def rec_unit(qi, db, recf, recs, last=False):
                nc.vector.reciprocal_approx_fast(recf, db)
                if last:
                    nc.scalar.copy(recs, recf)
                else:
                    nc.vector.tensor_copy(recs, recf)

            def norm_unit(qi, recs):
                pb2 = ps_pool.tile([128, QT], F32, tag="sc", name=f"pb{qi}")
                nc.tensor.matmul(
                    pb2[0:HD, :], ones64, recs[:, 0:QT], start=True, stop=True
                )
                nc.tensor.matmul(
                    pb2[HD:128, :], ones64, recs[:, QT : 2 * QT],
                    start=True, stop=True,
                )
                nc.vector.scalar_tensor_tensor(
                    aots[qi], pb2, 1.0, posbs[qi], op0=MUL, op1=MUL
                )

            outr = out.rearrange("(c p) d -> p c d", p=128)

            def fin_unit(qi, t2):
                # two 128-t output chunks per psum tile (halves the number
                # of 4KB insertions into the shared psum rotation)
                ps = ps_pool.tile(
                    [128, 2, D], F32, tag="sc", name=f"f{qi}_{t2}"
                )
                for u in range(2):
                    nc.tensor.matmul(
                        ps[:, u, :],
                        aots[qi][:, (2 * t2 + u) * 128 : (2 * t2 + u + 1) * 128],
                        wo_sb,
                        start=True,
                        stop=True,
                    )
                osb = osb_pool.tile([128, 2, D], F32, tag="osb")
                nc.scalar.copy(osb, ps)
                c0 = qi * 4 + 2 * t2
                nc.sync.dma_start(outr[:, c0 : c0 + 2, :], osb)

            posbs = {}
            po_drain = [None]

            # ---- attention ----
            for qi in range(NQT):
                qs = qi * QT
                aots[qi] = aot_pool.tile(
                    [128, QT], BF16, tag="aot", name=f"aot{qi}"
                )
                # po: both heads + denominators in one 2-bank psum tile
                po = po_pool.tile([65, 2 * QT], F32, tag="po", name=f"po{qi}")

                def scores(k, qs=qs):
                    pss = ps_pool.tile([128, 2, QT], F32, tag="sc")
                    for hh in range(2):
                        off = hh * HD
                        nc.tensor.matmul(
                            pss[:, hh, :],
                            kt[off : off + HD, k * 128 : (k + 1) * 128],
                            qt[off : off + HD, qs : qs + QT],
                            start=True,
                            stop=True,
                        )
                    return pss

                ptiles = {}

                def exp(k, pss):
                    cp, j = divmod(k, 2)
                    if j == 0:
                        ptiles[cp] = pt_pool.tile(
                            [128, 2, 2, QT], FP8, tag="pt", name=f"pt{cp}"
                        )
                    pt = ptiles[cp]
                    if k % 2 == 0 or k == 19:
                        # ACT: exact exp, fp8e4 convert
                        nc.scalar.activation(
                            pt[:, :, j, :], pss,
                            mybir.ActivationFunctionType.Exp, scale=0.125,
                        )
                    else:
                        # DVE: e4m3 bit-pattern exp (Schraudolph)
                        nc.vector.tensor_scalar(
                            pt[:, :, j, :].bitcast(U8), pss,
                            LOG2E, SCHRAU_C, op0=MUL, op1=ADD,
                        )

                def attn_v(cp, po=po):
                    pt = ptiles.pop(cp)
                    for hh in range(2):
                        nc.tensor.matmul(
                            po[:, hh * QT : (hh + 1) * QT],
                            v2r[:, cp, :, hh, 0:65],
                            pt[:, hh, :, :],
                            start=(cp == 0),
                            stop=(cp == NCP - 1),
                            perf_mode=DR,
                        )

                # software pipeline: scores/exp run ahead of attn@V
                pss = scores(0)
                exp(0, pss)
                if qi == 0:
                    k_unit(0, 128, QT)  # kt chunks 1-3, before scores(1)
                pss = scores(1)
                exp(1, pss)
                # drain the PREVIOUS qtile's po only now -- after this
                # qtile's first two exps are queued on ACT/DVE -- so
                # attn_v(0) (blocked on the po banks) unblocks while the
                # exps are already done rather than queued behind these.
                if po_drain[0] is not None:
                    po_drain[0]()
                    po_drain[0] = None
                for cp in range(NCP):
                    it = qi * NCP + cp
                    # drain deferred projection / norm / fin work FIRST so
                    # their engine-queue entries precede the scores/attn@V
                    # that consume them (k_unit(tt) must precede the
                    # lookahead scores(4tt) below; v_unit(j) must precede
                    # attn_v(j//2)).
                    if pending:
                        pending.pop(0)()
                        if pending:
                            pending.pop(0)()
                        if pending and len(pending) > 8:
                            pending.pop(0)()
                    elif not pending and pending_slow and it % 2 == 1:
                        pending_slow.pop(0)()
                    for k in (2 * cp + 2, 2 * cp + 3):
                        if k < KCH:
                            pss = scores(k)
                            exp(k, pss)
                    attn_v(cp)

                # free po fast: the four po-readers run two-per-engine in
                # parallel; the reciprocal is deferred off the boundary.
                db = small_pool.tile([1, 2 * QT], F32, tag="db")
                posbN = posb_pool.tile(
                    [128, QT], F32, tag="posb", name=f"posb{qi}"
                )

                def drain_po(qi=qi, po=po, db=db, posbN=posbN):
                    nc.scalar.copy(db[:, 0:QT], po[64:65, 0:QT])
                    nc.vector.tensor_copy(
                        db[:, QT : 2 * QT], po[64:65, QT : 2 * QT]
                    )
                    nc.vector.tensor_copy(posbN[0:HD, :], po[0:HD, 0:QT])
                    nc.scalar.copy(posbN[HD:128, :], po[0:HD, QT : 2 * QT])

                posbs[qi] = posbN
                recf = small_pool.tile([1, 2 * QT], F32, tag="recf")
                recs = small_pool.tile([1, 2 * QT], FP16, tag="rec")
                last = qi == NQT - 1
                if last:
                    drain_po()
                    rec_unit(qi, db, recf, recs, last=True)
                else:
                    po_drain[0] = drain_po
                    pending_slow.append(
                        lambda qi=qi, db=db, recf=recf, recs=recs: rec_unit(
                            qi, db, recf, recs
                        )
                    )
                pending_slow.append(lambda qi=qi, recs=recs: norm_unit(qi, recs))
                pending_slow.extend(
                    lambda qi=qi, t2=t2: fin_unit(qi, t2) for t2 in range(2)
                )

            for u in pending + pending_slow:
                u()

    nc.compile()
    return nc


_NC_CACHE = None


def _get_program():
    global _NC_CACHE
    if _NC_CACHE is None:
        _NC_CACHE = _build_program()
    return _NC_CACHE


def prepare_in_maps(x, Wq, bq, Wk, bk, Wv, bv, Wo, bo):
    bf = ml_dtypes.bfloat16
    x = np.ascontiguousarray(np.asarray(x, dtype=np.float32)).astype(bf)
    wqT = np.asarray(Wq, np.float32).T  # [D in, D out-rows]
    wkT = np.asarray(Wk, np.float32).T
    wvT = np.asarray(Wv, np.float32).T
    woT = np.asarray(Wo, np.float32).T  # [D dv, D out]
    bq = np.asarray(bq, np.float32)
    bk = np.asarray(bk, np.float32)
    bv = np.asarray(bv, np.float32)
    in_maps = []
    for core in range(NCORES):
        b = core // PAIRS
        hp = core % PAIRS
        pr = slice(hp * 128, (hp + 1) * 128)
        m = {
            "xt": np.ascontiguousarray(x[b].T),
            "wqt": np.ascontiguousarray(wqT[:, pr]).astype(bf),
            "wkt": np.ascontiguousarray(wkT[:, pr]).astype(bf),
            "wvt": np.ascontiguousarray(wvT[:, pr]).astype(bf),
            "wos": np.ascontiguousarray(woT[pr, :]).astype(bf),
            "bqs": np.ascontiguousarray(bq[pr].reshape(128, 1)),
            "bks": np.ascontiguousarray(bk[pr].reshape(128, 1)),
            "bvb": np.ascontiguousarray(
                np.broadcast_to(bv[pr][None, :], (128, 128))
            ),
        }
        in_maps.append(m)
    return in_maps


def assemble(results, bo):
    out = np.empty((B, S, D), dtype=np.float32)
    bo = np.asarray(bo, np.float32)
    for b in range(B):
        acc = results[b * PAIRS]["out"].astype(np.float32, copy=True)
        for hp in range(1, PAIRS):
            acc += results[b * PAIRS + hp]["out"]
        out[b] = acc + bo[None, :]
    return out


def kernel(x, Wq, bq, Wk, bk, Wv, bv, Wo, bo):
    in_maps = prepare_in_maps(x, Wq, bq, Wk, bk, Wv, bv, Wo, bo)
    nc = _get_program()
    res = run_bass_kernel_spmd(nc, in_maps, core_ids=list(range(NCORES)))
    return assemble(res.results, bo)
